# revision 1
# baseline (speedup 1.0000x reference)
"""Trainium2 Bass kernel for nn_BinaryMemoryRNN (scatter_memory).

Computation (reference):
    logits = h_prev @ Mw.T + Mb                 # [B, 28]
    b1/b2  = bits of logits halves (> 0)
    idx1   = clip(sum(b1 * 2^(13-j)), 0, 8191)
    idx2   = clip(sum(b2 * 2^(13-j)), 8192, 16383)
    pre    = x @ Ww.T + h_prev @ Uw.T + mem[idx1] @ Qrw.T + mem[idx2] @ Qlw.T + bias
    out    = sigmoid(layernorm(pre) * gamma + beta)

Strategy: data-parallel over batch across 8 cores (1024 rows each).
  - Activations pre-transposed on host to [feature, batch] layout (the PE
    contracts over the partition dim), bf16 for the 4 big matmuls.
  - logits matmul in fp32 (index bits are sign-sensitive).
  - memory table replicated in DRAM as bf16 [16384, 1024]; rows are fetched
    with gpsimd.dma_gather(transpose=True) which lands them directly in
    [feature, batch] layout.
  - LayerNorm + sigmoid epilogue on DVE/ACT per 128-row tile.
"""

import sys

sys.path.insert(0, "/opt/trn_rl_repo")

from contextlib import ExitStack

import numpy as np
import ml_dtypes

import concourse.bass as bass
import concourse.tile as tile
from concourse import bacc, mybir, library_config
from concourse.bass_utils import run_bass_kernel_spmd

F32 = mybir.dt.float32
BF16 = mybir.dt.bfloat16
I16 = mybir.dt.int16

B, I, H, NB = 8192, 1024, 1024, 14
MEM = 2**NB
NCORES = 8
BL = B // NCORES  # 1024 batch rows per core
KC = H // 128  # 8 contraction chunks
MT = BL // 128  # 8 output row-tiles per core
EPS = 1e-5

_CACHE = {}


def _build(trivial_gb: bool, dump_debug: bool = False, no_gather: bool = False):
    """Trace the Bass/Tile module (shared by all 8 cores, SPMD)."""
    nc = bacc.Bacc(
        "TRN2", target_bir_lowering=False, debug=False, enable_asserts=True
    )

    x_t = nc.dram_tensor("x_t", [128, KC, BL], BF16, kind="ExternalInput").ap()
    h_t32 = nc.dram_tensor("h_t32", [128, KC, BL], F32, kind="ExternalInput").ap()
    # weights, [src, feat_in(part), feat_in(chunk), feat_out]; src order W,U,Qr,Ql
    w_t = nc.dram_tensor("w_t", [4, 128, KC, H], BF16, kind="ExternalInput").ap()
    # packed consts: mw[0:224] | bias[224:1248] | pw-unused[1248:1250] |
    # clip[1250:1252] | negmb[1252:1253] | ident-as-f32[1253:1317] |
    # pw-as-bf16[1317:1318]
    NCONST = 1318
    const_t = nc.dram_tensor("const_t", [128, NCONST], F32, kind="ExternalInput").ap()
    mem_t = nc.dram_tensor("mem_t", [MEM, H], BF16, kind="ExternalInput").ap()
    if not trivial_gb:
        gam_t = nc.dram_tensor("gam_t", [128, H], F32, kind="ExternalInput").ap()
        bet_t = nc.dram_tensor("bet_t", [128, H], F32, kind="ExternalInput").ap()
    out_t = nc.dram_tensor("out_t", [BL, H], F32, kind="ExternalOutput").ap()
    if dump_debug:
        dbg_bits = nc.dram_tensor(
            "dbg_bits", [2 * NB, BL], F32, kind="ExternalOutput"
        ).ap()
        dbg_idx = nc.dram_tensor(
            "dbg_idx", [2, BL], I16, kind="ExternalOutput"
        ).ap()
        dbg_mem = nc.dram_tensor(
            "dbg_mem", [128, KC, BL], BF16, kind="ExternalOutput"
        ).ap()

    with tile.TileContext(nc) as tc:
        with ExitStack() as ctx:
            # ---------------- pools ----------------
            cpool = ctx.enter_context(tc.tile_pool(name="consts", bufs=1))
            apool = ctx.enter_context(tc.tile_pool(name="acts", bufs=1))
            # h32 halves and raw gathered tiles share 16KB/partition slots:
            # h32 dies after the logits matmul, before the gathers land.
            hpool = ctx.enter_context(tc.tile_pool(name="h32_or_gather", bufs=4))
            gpool = ctx.enter_context(tc.tile_pool(name="gathered", bufs=1))
            spool = ctx.enter_context(tc.tile_pool(name="small", bufs=2))
            epool = ctx.enter_context(tc.tile_pool(name="epilogue", bufs=2))
            pp_main = ctx.enter_context(
                tc.tile_pool(name="psum_main", bufs=2, space="PSUM")
            )
            # logits / idx / PE-transpose outputs share two 2-bank slots
            pp_small = ctx.enter_context(
                tc.tile_pool(name="psum_small", bufs=2, space="PSUM")
            )

            # gpsimd ucode library containing DMAGatherAnt; load it up front
            # so the Q7 IRAM reload overlaps the initial DMAs.
            nc.gpsimd.load_library(library_config.attnmlp)

            # ---------------- input loads ----------------
            # critical path first: packed consts + h fp32 for the index pipeline
            const_sb = cpool.tile([128, NCONST], F32, tag="const")
            nc.sync.dma_start(const_sb[:], const_t[:])
            mw_sb = const_sb[:, 0:224].rearrange("p (k j) -> p k j", j=2 * NB)
            bias_sb = const_sb[:, 224:1248]
            pw_sb = const_sb[0 : 2 * NB, 1317:1318].bitcast(BF16)
            clip_sb = const_sb[0:2, 1250:1252]
            negmb_sb = const_sb[0 : 2 * NB, 1252:1253]
            ident_sb = const_sb[:, 1253:1317].bitcast(BF16)
            eps_sb = cpool.tile([128, 1], F32, tag="eps")
            nc.vector.memset(eps_sb[:], EPS)

            # h32 split 1/2/2/2/1 chunks: the logits matmul starts after the
            # first 512KB, and the 8KB middle pieces share pool slots with
            # the half-gather destinations later
            h32_k0 = spool.tile([128, 1, BL], F32, tag="h32k0")
            nc.sync.dma_start(h32_k0[:], h_t32[:, 0:1, :])
            h32_mid = []
            for piece in range(3):
                hp = hpool.tile([128, 2, BL], F32, tag="slab")
                nc.sync.dma_start(
                    hp[:], h_t32[:, 1 + 2 * piece : 3 + 2 * piece, :]
                )
                h32_mid.append(hp)
            h32_k7 = spool.tile([128, 1, BL], F32, tag="h32k7")
            nc.sync.dma_start(h32_k7[:], h_t32[:, KC - 1 : KC, :])

            def h32_chunk(k):
                if k == 0:
                    return h32_k0[:, 0, :]
                if k == KC - 1:
                    return h32_k7[:, 0, :]
                return h32_mid[(k - 1) // 2][:, (k - 1) % 2, :]

            x_sb = apool.tile([128, KC, BL], BF16, tag="x")
            nc.sync.dma_start(x_sb[:], x_t[:])
            # h16 derived on-device from h32 (gpsimd is idle; saves a 2MB load)
            h16_sb = apool.tile([128, KC, BL], BF16, tag="h16")
            nc.gpsimd.tensor_copy(h16_sb[:, 0:1, :], h32_k0[:])
            for piece in range(3):
                nc.gpsimd.tensor_copy(
                    h16_sb[:, 1 + 2 * piece : 3 + 2 * piece, :], h32_mid[piece][:]
                )
            nc.gpsimd.tensor_copy(h16_sb[:, KC - 1 : KC, :], h32_k7[:])
            # W and U weights now; Qr/Ql weights are loaded later so they
            # don't queue ahead of latency-critical small DMAs
            w_sb = []
            for s in range(4):
                w = cpool.tile([128, KC, H], BF16, tag=f"w{s}")
                if s < 2:
                    nc.sync.dma_start(w[:], w_t[s])
                w_sb.append(w)

            def act_slice(tiles, k, sl):
                return tiles[:, k, sl]
            if not trivial_gb:
                gam_sb = cpool.tile([128, H], F32, tag="gam")
                nc.sync.dma_start(gam_sb[:], gam_t[:])
                bet_sb = cpool.tile([128, H], F32, tag="bet")
                nc.sync.dma_start(bet_sb[:], bet_t[:])

            # ---------------- index pipeline ----------------
            # logits.T [28, BL] fp32, accumulated over KC chunks; k-outer so
            # the first half of h32 is enough to start
            logit_ps = pp_small.tile([2 * NB, BL], F32, tag="sm")
            for k in range(KC):
                hk = h32_chunk(k)
                for n in range(BL // 512):
                    nc.tensor.matmul(
                        logit_ps[:, n * 512 : (n + 1) * 512],
                        mw_sb[:, k, :],
                        hk[:, n * 512 : (n + 1) * 512],
                        start=(k == 0),
                        stop=(k == KC - 1),
                    )
            # bits = (h@Mw.T + Mb > 0)  <=>  (h@Mw.T > -Mb), as 1.0/0.0
            # (bf16: exact for 0/1, and the powers matmul sums stay exact
            # in fp32 PSUM)
            bits_sb = spool.tile([2 * NB, BL], BF16, tag="bits")
            nc.vector.tensor_scalar(
                bits_sb[:], logit_ps[:], negmb_sb[:, 0:1], None,
                mybir.AluOpType.is_gt,
            )
            # raw indices via tiny matmul with powers of two: [2, BL]
            idx_ps = pp_small.tile([2, BL], F32, tag="sm")
            for n in range(BL // 512):
                nc.tensor.matmul(
                    idx_ps[:, n * 512 : (n + 1) * 512],
                    pw_sb,
                    bits_sb[:, n * 512 : (n + 1) * 512],
                    start=True,
                    stop=True,
                )
            # clip + cast to int16 (values are exact integers in fp32);
            # per-partition clip bounds: row0 -> [0, 8191], row1 -> [8192, 16383]
            idx16 = spool.tile([2, BL], I16, tag="idx16")
            nc.vector.tensor_scalar(
                idx16[:], idx_ps[:], clip_sb[:, 0:1], clip_sb[:, 1:2],
                mybir.AluOpType.max, mybir.AluOpType.min,
            )

            # Wrap each index row into the [16, BL/16] layout dma_gather wants,
            # replicated to every 16-partition group (the Q7 ucode cores each
            # read their own group). Stage S[i, 32j+q'] = idx[(32j+i)*16+q'%16]
            # (16 columns duplicated within each 32-block), then four DVE
            # 32x32 block-transposes to partition bases 0/32/64/96.
            idxw_r = []
            for r in range(2):
                # issue on ACT's HWDGE FIFO so this tiny latency-critical
                # transfer doesn't queue behind the big input loads on SP's;
                # strided DMAs fill cols {0:16, 32:48}, a DVE copy
                # duplicates into cols {16:32, 48:64}
                stg = spool.tile([32, 64], I16, tag="stage")
                stg_j = stg[0:32, :].rearrange("p (j hq) -> p j hq", j=2)
                with nc.allow_non_contiguous_dma(reason="tiny idx wrap staging"):
                    for j in range(2):
                        nc.scalar.dma_start(
                            stg[0:32, 32 * j : 32 * j + 16],
                            idx16[r : r + 1, j * 512 : (j + 1) * 512].rearrange(
                                "p (a b) -> p a b", b=16
                            ),
                        )
                nc.vector.tensor_copy(stg_j[:, :, 16:32], stg_j[:, :, 0:16])
                idxw = spool.tile([128, 64], I16, tag="idxw")
                for g in range(4):
                    nc.vector.transpose(idxw[32 * g : 32 * (g + 1), :], stg[:])
                idxw_r.append(idxw)

            # gathers split in batch halves, interleaved r0/r1, so blocks
            # c=0-3 of BOTH tensors arrive after the first two half-gathers.
            # g2[r][hf][p, c, :] = mem[idx_{(4*hf+c)*128+p}, :]
            HB = BL // 2
            g2_tiles = [[None, None], [None, None]]
            for hf in range(2):
                for r in range(2):
                    g2 = hpool.tile([128, HB // 128, H], BF16, tag="slab")
                    if no_gather:
                        nc.sync.dma_start(
                            g2[:],
                            mem_t.rearrange("(a p) h -> p a h", p=128)[
                                :, 0 : HB // 128, :
                            ],
                        )
                    else:
                        nc.gpsimd.dma_gather(
                            out_ap=g2[:],
                            in_ap=mem_t[:],
                            idxs_ap=idxw_r[r][:, hf * 32 : (hf + 1) * 32],
                            num_idxs=HB,
                            num_idxs_reg=HB,
                            elem_size=H,
                            transpose=False,
                        )
                    g2_tiles[r][hf] = g2

            # Qr/Ql weights: needed only once the mem matmuls start
            for s in (2, 3):
                nc.sync.dma_start(w_sb[s][:], w_t[s])

            # ---------------- main matmuls + epilogue ----------------
            # Emission order = PE stream order: x/h matmuls for the first two
            # row-tiles run while the gather is in flight; then PE-transposes
            # of the gathered rows; then the mem matmuls + epilogues pipeline
            # with the remaining x/h matmuls.
            srcs_xh = [(x_sb, 0), (h16_sb, 1)]
            ps_tiles = {}

            def emit_xh(m):
                ps = pp_main.tile([128, H], F32, tag="acc")
                ps_tiles[m] = ps
                ms = slice(m * 128, (m + 1) * 128)
                for si, (act, wi) in enumerate(srcs_xh):
                    for k in range(KC):
                        lhs = act_slice(act, k, ms)
                        for n in range(H // 512):
                            nc.tensor.matmul(
                                ps[:, n * 512 : (n + 1) * 512],
                                lhs,
                                act_slice(
                                    w_sb[wi], k, slice(n * 512, (n + 1) * 512)
                                ),
                                start=(si == 0 and k == 0),
                                stop=False,
                            )

            def emit_mem_epilogue(m):
                ps = ps_tiles.pop(m)
                ms = slice(m * 128, (m + 1) * 128)
                for si in range(2):
                    mt = mem_sb[si][m]  # [128, KC, 128] block for this m
                    for k in range(KC):
                        lhs = mt[:, k, :]
                        for n in range(H // 512):
                            nc.tensor.matmul(
                                ps[:, n * 512 : (n + 1) * 512],
                                lhs,
                                act_slice(
                                    w_sb[2 + si], k, slice(n * 512, (n + 1) * 512)
                                ),
                                start=False,
                                stop=(si == 1 and k == KC - 1),
                            )

                # t = pre + bias  (bias varies along the free/feature dim)
                t = epool.tile([128, H], F32, tag="t")
                nc.vector.tensor_tensor(
                    t[:], ps[:], bias_sb[:], mybir.AluOpType.add
                )
                # layernorm stats
                st6 = epool.tile([128, 2, 6], F32, tag="st6")
                for a in range(2):
                    nc.vector.bn_stats(st6[:, a, :], t[:, a * 512 : (a + 1) * 512])
                mv = epool.tile([128, 2], F32, tag="mv")
                nc.vector.bn_aggr(mv[:], st6.rearrange("p a b -> p (a b)"))
                # rstd = 1/sqrt(var + eps): ACT sqrt, then the fast custom-DVE
                # reciprocal (~18 correct bits, plenty for layernorm).
                # sc[:,0] holds std then -mu*rstd; sc[:,1] holds rstd.
                sc = epool.tile([128, 2], F32, tag="sc")
                nc.scalar.activation(
                    sc[:, 0:1], mv[:, 1:2], mybir.ActivationFunctionType.Sqrt,
                    bias=eps_sb[:, 0:1],
                )
                nc.vector.reciprocal_approx_fast(sc[:, 1:2], sc[:, 0:1])
                nc.vector.tensor_scalar(
                    sc[:, 0:1], mv[:, 0:1], sc[:, 1:2], -1.0,
                    mybir.AluOpType.mult, mybir.AluOpType.mult,
                )
                rstd = sc[:, 1:2]
                nmu = sc[:, 0:1]
                o = epool.tile([128, H], F32, tag="o")
                if trivial_gb:
                    # out = sigmoid((t - mu) * rstd)
                    nc.scalar.activation(
                        o[:], t[:], mybir.ActivationFunctionType.Sigmoid,
                        bias=nmu[:, 0:1], scale=rstd[:, 0:1],
                    )
                else:
                    xh = epool.tile([128, H], F32, tag="xh")
                    nc.scalar.activation(
                        xh[:], t[:], mybir.ActivationFunctionType.Identity,
                        bias=nmu[:, 0:1], scale=rstd[:, 0:1],
                    )
                    nc.vector.tensor_tensor(
                        xh[:], xh[:], gam_sb[:], mybir.AluOpType.mult
                    )
                    nc.vector.tensor_tensor(
                        xh[:], xh[:], bet_sb[:], mybir.AluOpType.add
                    )
                    zero_sb = cpool.tile([128, 1], F32, tag="zero")
                    nc.vector.memset(zero_sb[:], 0.0)
                    nc.scalar.activation(
                        o[:], xh[:], mybir.ActivationFunctionType.Sigmoid,
                        bias=zero_sb[:, 0:1],
                    )
                nc.sync.dma_start(out_t[ms, :], o[:])

            emit_xh(0)
            emit_xh(1)

            # PE-transpose gathered rows into [feat, batch] layout; one tile
            # per (tensor, batch-block) so each m-tile's mem matmuls depend
            # only on its own block's transposes
            mem_sb = [[], []]
            for c in range(BL // 128):
                for r in range(2):
                    g2 = g2_tiles[r][c // 4]
                    cc = c % 4
                    mt = gpool.tile([128, KC, 128], BF16, tag=f"mem{r}_{c}")
                    for k in range(KC):
                        tp = pp_small.tile([128, 128], BF16, tag="sm")
                        nc.tensor.transpose(
                            tp[:], g2[:, cc, k * 128 : (k + 1) * 128], ident_sb[:]
                        )
                        nc.vector.tensor_copy(mt[:, k, :], tp[:])
                    mem_sb[r].append(mt)

            if dump_debug:
                nc.sync.dma_start(dbg_bits[:], bits_sb[:])
                nc.sync.dma_start(dbg_idx[:], idx16[:])
                for c in range(BL // 128):
                    nc.sync.dma_start(
                        dbg_mem[:, :, c * 128 : (c + 1) * 128], mem_sb[0][c][:]
                    )

            emit_mem_epilogue(0)
            for m in range(2, MT):
                emit_xh(m)
                emit_mem_epilogue(m - 1)
            emit_mem_epilogue(MT - 1)

    nc.compile()  # bacc register allocation / DCE
    return nc


def _to_kxp(a, dtype):
    """[batch, feat] -> [128, KC, batch] with feat = k*128 + p."""
    t = np.ascontiguousarray(a.T.reshape(KC, 128, -1).transpose(1, 0, 2))
    return t.astype(dtype)


def prep(inputs):
    """Host-side shard/layout prep. Returns (in_maps, trivial_gb)."""
    x = np.asarray(inputs["x"], np.float32)
    h = np.asarray(inputs["h_prev"], np.float32)
    memory = np.asarray(inputs["memory"], np.float32)
    gamma = np.asarray(inputs["gamma"], np.float32)
    beta = np.asarray(inputs["beta"], np.float32)
    trivial_gb = bool(np.all(gamma == 1.0) and np.all(beta == 0.0))

    bf = ml_dtypes.bfloat16
    # W is [out, in]; the kernel wants w[p, k, n] = W[n, k*128+p], which is
    # exactly _to_kxp applied to W with (out, in) in the (batch, feat) slots.
    w_cat = np.stack(
        [_to_kxp(np.asarray(inputs[n], np.float32), bf) for n in ("Ww", "Uw", "Qrw", "Qlw")]
    )
    mw = _to_kxp(np.asarray(inputs["Mw"], np.float32), np.float32)  # [128, KC, 28]

    pw = np.zeros((2 * NB, 2), np.float32)
    pw[:NB, 0] = 2.0 ** np.arange(NB - 1, -1, -1)
    pw[NB:, 1] = 2.0 ** np.arange(NB - 1, -1, -1)
    clip = np.array(
        [[0.0, MEM // 2 - 1], [MEM // 2, MEM - 1]], np.float32
    )  # [row, (lo, hi)]

    mem_bf = memory.astype(bf)
    ident = np.eye(128, dtype=np.float32).astype(bf)
    bias = (
        np.asarray(inputs["Wb"], np.float32)
        + np.asarray(inputs["Ub"], np.float32)
        + np.asarray(inputs["Qrb"], np.float32)
        + np.asarray(inputs["Qlb"], np.float32)
    )

    # pack all small constants into one [128, 1318] f32 buffer (single DMA)
    const = np.zeros((128, 1318), np.float32)
    const[:, 0:224] = mw.reshape(128, 224)
    const[:, 224:1248] = np.broadcast_to(bias, (128, H))
    const[: 2 * NB, 1248:1250] = pw
    const[:2, 1250:1252] = clip
    const[: 2 * NB, 1252:1253] = -np.asarray(inputs["Mb"], np.float32).reshape(
        2 * NB, 1
    )
    const[:, 1253:1317] = ident.view(np.float32)
    const[: 2 * NB, 1317:1318] = pw.astype(bf).view(np.float32)

    common = dict(w_t=w_cat, const_t=const, mem_t=mem_bf)
    if not trivial_gb:
        common["gam_t"] = np.ascontiguousarray(np.broadcast_to(gamma, (128, H)))
        common["bet_t"] = np.ascontiguousarray(np.broadcast_to(beta, (128, H)))

    in_maps = []
    for c in range(NCORES):
        xs = x[c * BL : (c + 1) * BL]
        hs = h[c * BL : (c + 1) * BL]
        in_maps.append(
            dict(x_t=_to_kxp(xs, bf), h_t32=_to_kxp(hs, np.float32), **common)
        )
    return in_maps, trivial_gb


def get_nc(trivial_gb):
    key = ("nc", trivial_gb)
    if key not in _CACHE:
        _CACHE[key] = _build(trivial_gb)
    return _CACHE[key]


def run(inputs, trace=False, **kw):
    in_maps, trivial_gb = prep(inputs)
    nc = get_nc(trivial_gb)
    res = run_bass_kernel_spmd(
        nc, in_maps, core_ids=list(range(NCORES)), trace=trace, **kw
    )
    out = np.concatenate([res.results[c]["out_t"] for c in range(NCORES)], axis=0)
    return out.astype(np.float32), res


def kernel(**inputs):
    return run(inputs)[0]



# revision 9
# speedup vs baseline: 1.6628x; 1.6628x over previous
"""Trainium2 Bass kernel for nn_BinaryMemoryRNN (scatter_memory).

Computation (reference):
    logits = h_prev @ Mw.T + Mb                 # [B, 28]
    b1/b2  = bits of logits halves (> 0)
    idx1   = clip(sum(b1 * 2^(13-j)), 0, 8191)
    idx2   = clip(sum(b2 * 2^(13-j)), 8192, 16383)
    pre    = x @ Ww.T + h_prev @ Uw.T + mem[idx1] @ Qrw.T + mem[idx2] @ Qlw.T + b
    out    = sigmoid(layernorm(pre) * gamma + beta)

Strategy: data-parallel over batch across 8 cores (1024 rows each).
  - All four big matmuls in fp8 (e4m3) with DoubleRow (2x PE throughput).
    Weights scaled by 512 (LayerNorm is scale-invariant, bias scaled too).
  - Memory table stored centered (mem - 0.5) in fp8; the 0.5*rowsum(Q)
    correction is folded into the bias. Rows are fetched with
    gpsimd.dma_gather(transpose=True), which lands them 16-bit-granular
    interleaved in [feature, batch] layout; the byte-interleaved pairs are
    consumed directly by DoubleRowSwInterleave matmuls (no PE transposes).
    SWInterleave reads stationary columns reversed; this is absorbed by
    staging x8/h8 with each 128-batch block reversed on the host and
    un-reversing the output on the host.
  - logits matmul in fp32 (index bits are sign-sensitive).
  - Epilogue: bias add + bn_stats on DVE, rstd via quake-rsqrt bit trick on
    DVE (avoids ACT sqrt<->sigmoid table thrash), single fused
    scale+bias+Sigmoid on ACT, bf16 output.
  - PE HAM warmup matmuls at t0 so the index pipeline runs at 2.4 GHz.
"""

import sys

sys.path.insert(0, "/opt/trn_rl_repo")

from contextlib import ExitStack

import numpy as np
import ml_dtypes

import concourse.bass as bass
import concourse.tile as tile
from concourse import bacc, mybir, library_config
from concourse.bass_utils import run_bass_kernel_spmd

F32 = mybir.dt.float32
BF16 = mybir.dt.bfloat16
F8 = mybir.dt.float8e4
I16 = mybir.dt.int16
I32 = mybir.dt.int32
f8np = ml_dtypes.float8_e4m3fn
bfnp = ml_dtypes.bfloat16

B, I, H, NB = 8192, 1024, 1024, 14
MEM = 2**NB
NCORES = 8
BL = B // NCORES  # 1024 batch rows per core
KC = H // 128  # 8 contraction chunks of 128
MT = BL // 128  # 8 output row-tiles per core
EPS = 1e-5
WSCALE = 512.0
EPS_S = EPS * WSCALE * WSCALE
QUAKE = 0x5F3759DF

_CACHE = {}


def _build(trivial_gb: bool):
    nc = bacc.Bacc(
        "TRN2", target_bir_lowering=False, debug=False, enable_asserts=True
    )

    # activations: feat-major [128, KC, BL]; x8/h8 batch-reversed per 128-block
    x8_t = nc.dram_tensor("x8_t", [128, KC, BL], F8, kind="ExternalInput").ap()
    h8_t = nc.dram_tensor("h8_t", [128, KC, BL], F8, kind="ExternalInput").ap()
    h32_t = nc.dram_tensor("h32_t", [128, KC, BL], F32, kind="ExternalInput").ap()
    # W/U weights: [2, 128, KC, H] fp8, w[s][p,k,n] = Ws[n, 128k+p]*S
    wxu_t = nc.dram_tensor("wxu_t", [2, 128, KC, H], F8, kind="ExternalInput").ap()
    # Qr/Ql weights: [2, 128, 4, 2, H] fp8, w[s][p,c,b,n] = Qs[n, 2*(128c+p)+b]*S
    wq_t = nc.dram_tensor("wq_t", [2, 128, 4, 2, H], F8, kind="ExternalInput").ap()
    # packed consts: mw[0:224] | bias[224:1248] | clip[1248:1250] |
    # negmb[1250:1251] | pw-as-bf16[1251:1252]
    NCONST = 1252
    const_t = nc.dram_tensor("const_t", [128, NCONST], F32, kind="ExternalInput").ap()
    mem_t = nc.dram_tensor("mem_t", [MEM, H], F8, kind="ExternalInput").ap()
    if not trivial_gb:
        gam_t = nc.dram_tensor("gam_t", [128, H], F32, kind="ExternalInput").ap()
        bet_t = nc.dram_tensor("bet_t", [128, H], F32, kind="ExternalInput").ap()
    out_t = nc.dram_tensor("out_t", [BL, H], BF16, kind="ExternalOutput").ap()

    DR = mybir.MatmulPerfMode.DoubleRow
    DRI = mybir.MatmulPerfMode.DoubleRowSwInterleave

    with tile.TileContext(nc) as tc:
        with ExitStack() as ctx:
            cpool = ctx.enter_context(tc.tile_pool(name="consts", bufs=1))
            apool = ctx.enter_context(tc.tile_pool(name="acts", bufs=1))
            spool = ctx.enter_context(tc.tile_pool(name="small", bufs=2))
            epool = ctx.enter_context(tc.tile_pool(name="epilogue", bufs=2))
            pp_main = ctx.enter_context(
                tc.tile_pool(name="psum_main", bufs=3, space="PSUM")
            )
            pp_small = ctx.enter_context(
                tc.tile_pool(name="psum_small", bufs=1, space="PSUM")
            )

            nc.gpsimd.load_library(library_config.attnmlp)

            # ---------------- PE warmup + ACT table prefetch ----------------
            warm_sb = cpool.tile([128, 512], BF16, tag="warm")
            nc.vector.memset(warm_sb[:], 0.0)
            eps_sb = cpool.tile([128, 2], F32, tag="eps")
            nc.vector.memset(eps_sb[:, 0:1], EPS_S)
            nc.vector.memset(eps_sb[:, 1:2], 0.0)
            # prefetch the sigmoid table set while DMAs run
            dum_sb = cpool.tile([128, 1], F32, tag="dum")
            nc.vector.memset(dum_sb[:], 0.0)
            nc.scalar.activation(
                dum_sb[:], dum_sb[:], mybir.ActivationFunctionType.Sigmoid
            )
            for w in range(40):
                wps = pp_main.tile([128, 512], F32, tag="acc", name=f"wm{w}")
                nc.tensor.matmul(
                    wps[:], warm_sb[:, 0:128], warm_sb[:], start=True, stop=True,
                )

            # ---------------- input loads ----------------
            const_sb = cpool.tile([128, NCONST], F32, tag="const")
            nc.sync.dma_start(const_sb[:], const_t[:])
            mw_sb = const_sb[:, 0:224].rearrange("p (k j) -> p k j", j=2 * NB)
            bias_sb = const_sb[:, 224:1248]
            clip_sb = const_sb[0:2, 1248:1250]
            negmb_sb = const_sb[0 : 2 * NB, 1250:1251]
            pw_sb = const_sb[0 : 2 * NB, 1251:1252].bitcast(BF16)

            # h32 split 1/2/2/2/1 so the logits matmul starts on chunk 0 early
            h32_k0 = spool.tile([128, 1, BL], F32, tag="h32k0")
            nc.sync.dma_start(h32_k0[:], h32_t[:, 0:1, :])
            h32_mid = []
            for piece in range(3):
                hp = spool.tile([128, 2, BL], F32, tag=f"h32m{piece}")
                nc.sync.dma_start(hp[:], h32_t[:, 1 + 2 * piece : 3 + 2 * piece, :])
                h32_mid.append(hp)
            h32_k7 = spool.tile([128, 1, BL], F32, tag="h32k7")
            nc.sync.dma_start(h32_k7[:], h32_t[:, KC - 1 : KC, :])

            def h32_chunk(k):
                if k == 0:
                    return h32_k0[:, 0, :]
                if k == KC - 1:
                    return h32_k7[:, 0, :]
                return h32_mid[(k - 1) // 2][:, (k - 1) % 2, :]

            x8_sb = apool.tile([128, KC, BL], F8, tag="x8")
            nc.sync.dma_start(x8_sb[:], x8_t[:])
            h8_sb = apool.tile([128, KC, BL], F8, tag="h8")
            nc.sync.dma_start(h8_sb[:], h8_t[:])
            wxu_sb = []
            for s in range(2):
                w = cpool.tile([128, KC, H], F8, tag=f"wxu{s}")
                nc.sync.dma_start(w[:], wxu_t[s])
                wxu_sb.append(w)
            wq_sb = []
            for s in range(2):
                w = cpool.tile([128, 4, 2, H], F8, tag=f"wq{s}")
                nc.sync.dma_start(w[:], wq_t[s])
                wq_sb.append(w)
            if not trivial_gb:
                gam_sb = cpool.tile([128, H], F32, tag="gam")
                nc.sync.dma_start(gam_sb[:], gam_t[:])
                bet_sb = cpool.tile([128, H], F32, tag="bet")
                nc.sync.dma_start(bet_sb[:], bet_t[:])

            # ---------------- index pipeline (fp32, as reference) ----------
            # logits.T [28, BL] in two 512-column PSUM halves, k-outer
            logit_ps = pp_small.tile([2 * NB, BL], F32, tag="sm")
            for k in range(KC):
                hk = h32_chunk(k)
                for n in range(2):
                    nc.tensor.matmul(
                        logit_ps[:, n * 512 : (n + 1) * 512],
                        mw_sb[:, k, :],
                        hk[:, n * 512 : (n + 1) * 512],
                        start=(k == 0),
                        stop=(k == KC - 1),
                    )
            # bits = (h@Mw.T > -Mb) as 1.0/0.0 bf16
            bits_sb = spool.tile([2 * NB, BL], BF16, tag="bits")
            nc.vector.tensor_scalar(
                bits_sb[:], logit_ps[:], negmb_sb[:, 0:1], None,
                mybir.AluOpType.is_gt,
            )
            idx_ps = pp_small.tile([2, BL], F32, tag="sm")
            for n in range(2):
                nc.tensor.matmul(
                    idx_ps[:, n * 512 : (n + 1) * 512],
                    pw_sb,
                    bits_sb[:, n * 512 : (n + 1) * 512],
                    start=True,
                    stop=True,
                )
            idx16 = spool.tile([2, BL], I16, tag="idx16")
            nc.vector.tensor_scalar(
                idx16[:], idx_ps[:], clip_sb[:, 0:1], clip_sb[:, 1:2],
                mybir.AluOpType.max, mybir.AluOpType.min,
            )

            # wrap idx rows into the [16, n/16] layout dma_gather wants,
            # replicated to every 16-partition group (baseline machinery)
            idxw_r = []
            for r in range(2):
                stg = spool.tile([32, 64], I16, tag="stage")
                stg_j = stg[0:32, :].rearrange("p (j hq) -> p j hq", j=2)
                with nc.allow_non_contiguous_dma(reason="tiny idx wrap staging"):
                    for j in range(2):
                        nc.scalar.dma_start(
                            stg[0:32, 32 * j : 32 * j + 16],
                            idx16[r : r + 1, j * 512 : (j + 1) * 512].rearrange(
                                "p (a b) -> p a b", b=16
                            ),
                        )
                nc.vector.tensor_copy(stg_j[:, :, 16:32], stg_j[:, :, 0:16])
                idxw = spool.tile([128, 64], I16, tag="idxw")
                for g in range(4):
                    nc.vector.transpose(idxw[32 * g : 32 * (g + 1), :], stg[:])
                idxw_r.append(idxw)

            # transposed gathers: fp8 rows land [128, 4, 2*512] 16-bit-granular
            # interleaved: partition p, chunk c, byte 2i+b = feature
            # 2*(128c+p)+b of gathered row i. Interleaved (hf, r) so m-tiles
            # 0-3 of both tensors unblock first.
            HB = BL // 2
            g_tiles = [[None, None], [None, None]]  # [r][hf]
            for hf in range(2):
                for r in range(2):
                    g8 = spool.tile([128, 8, HB], F8, tag=f"g{r}{hf}")
                    nc.gpsimd.dma_gather(
                        out_ap=g8[:],
                        in_ap=mem_t[:],
                        idxs_ap=idxw_r[r][:, hf * 32 : (hf + 1) * 32],
                        num_idxs=HB,
                        num_idxs_reg=HB,
                        elem_size=H,
                        transpose=True,
                    )
                    g_tiles[r][hf] = g8[:].rearrange("p (c t) i -> p c (t i)", t=2)

            # ---------------- main matmuls + epilogue ----------------
            ps_tiles = {}

            def emit_xh(m):
                ps = pp_main.tile([128, H], F32, tag="acc")
                ps_tiles[m] = ps
                ms = slice(m * 128, (m + 1) * 128)
                for si, act in enumerate((x8_sb, h8_sb)):
                    for kp in range(KC // 2):
                        lhsT = act[:, 2 * kp : 2 * kp + 2, ms]
                        for n in range(2):
                            nc.tensor.matmul(
                                ps[:, n * 512 : (n + 1) * 512],
                                lhsT,
                                wxu_sb[si][:, 2 * kp : 2 * kp + 2,
                                           n * 512 : (n + 1) * 512],
                                start=(si == 0 and kp == 0),
                                stop=False,
                                perf_mode=DR,
                            )

            def emit_mem_epilogue(m):
                ps = ps_tiles.pop(m)
                ms = slice(m * 128, (m + 1) * 128)
                mm = m % 4
                for si in range(2):
                    g_v = g_tiles[si][m // 4]
                    for c in range(4):
                        lhsT = g_v[:, c, 2 * 128 * mm : 2 * 128 * (mm + 1)]
                        for n in range(2):
                            nc.tensor.matmul(
                                ps[:, n * 512 : (n + 1) * 512],
                                lhsT,
                                wq_sb[si][:, c, :, n * 512 : (n + 1) * 512],
                                start=False,
                                stop=(si == 1 and c == 3),
                                perf_mode=DRI,
                            )

                # t = pre + bias (bias varies along the free/feature dim)
                t = epool.tile([128, H], BF16, tag="t")
                nc.vector.tensor_tensor(
                    t[:], ps[:], bias_sb[:], mybir.AluOpType.add
                )
                # layernorm stats
                st6 = epool.tile([128, 2, 6], F32, tag="st6")
                for a in range(2):
                    nc.vector.bn_stats(st6[:, a, :], t[:, a * 512 : (a + 1) * 512])
                mv = epool.tile([128, 2], F32, tag="mv")
                nc.vector.bn_aggr(mv[:], st6.rearrange("p a b -> p (a b)"))
                # rstd = 1/sqrt(var + eps) via quake bit trick + 1 Newton
                # (max rel err ~1.8e-3; all on DVE, no ACT table swap)
                sc = epool.tile([128, 4], F32, tag="sc")
                v = sc[:, 0:1]
                nc.vector.tensor_scalar(
                    v, mv[:, 1:2], eps_sb[:, 0:1], None, mybir.AluOpType.add
                )
                y0i = sc[:, 1:2].bitcast(I32)
                nc.vector.tensor_scalar(
                    y0i, v.bitcast(I32), 1, None,
                    mybir.AluOpType.logical_shift_right,
                )
                nc.vector.tensor_scalar(
                    y0i, y0i, -1, QUAKE,
                    mybir.AluOpType.mult, mybir.AluOpType.add,
                )
                y0 = sc[:, 1:2]
                a_t = sc[:, 2:3]
                nc.vector.tensor_tensor(a_t, y0, y0, mybir.AluOpType.mult)
                nc.vector.tensor_tensor(a_t, a_t, v, mybir.AluOpType.mult)
                nc.vector.tensor_scalar(
                    a_t, a_t, -0.5, 1.5, mybir.AluOpType.mult, mybir.AluOpType.add
                )
                rstd = sc[:, 3:4]
                nc.vector.tensor_tensor(rstd, y0, a_t, mybir.AluOpType.mult)
                # nmu = -mu * rstd
                nmu = sc[:, 1:2]
                nc.vector.tensor_scalar(
                    nmu, mv[:, 0:1], rstd, -1.0,
                    mybir.AluOpType.mult, mybir.AluOpType.mult,
                )
                o = epool.tile([128, H], BF16, tag="o")
                if trivial_gb:
                    nc.scalar.activation(
                        o[:], t[:], mybir.ActivationFunctionType.Sigmoid,
                        bias=nmu, scale=rstd,
                    )
                else:
                    xh = epool.tile([128, H], F32, tag="xh")
                    nc.scalar.activation(
                        xh[:], t[:], mybir.ActivationFunctionType.Identity,
                        bias=nmu, scale=rstd,
                    )
                    nc.vector.tensor_tensor(
                        xh[:], xh[:], gam_sb[:], mybir.AluOpType.mult
                    )
                    nc.vector.tensor_tensor(
                        xh[:], xh[:], bet_sb[:], mybir.AluOpType.add
                    )
                    nc.scalar.activation(
                        o[:], xh[:], mybir.ActivationFunctionType.Sigmoid,
                        bias=eps_sb[:, 1:2],
                    )
                nc.sync.dma_start(out_t[ms, :], o[:])

            emit_xh(0)
            emit_xh(1)
            emit_xh(2)
            emit_mem_epilogue(0)
            for m in range(3, MT):
                emit_xh(m)
                emit_mem_epilogue(m - 2)
            emit_mem_epilogue(MT - 2)
            emit_mem_epilogue(MT - 1)

    nc.compile()
    return nc


def _to_kxp(a, dtype):
    """[batch, feat] -> [128, KC, batch] with feat = k*128 + p."""
    t = np.ascontiguousarray(a.T.reshape(KC, 128, -1).transpose(1, 0, 2))
    return t.astype(dtype)


def _rev_blocks(a):
    """Reverse each 128-row block along the batch dim of [batch, feat]."""
    return np.ascontiguousarray(
        a.reshape(-1, 128, a.shape[-1])[:, ::-1, :].reshape(a.shape)
    )


def prep(inputs):
    """Host-side shard/layout prep. Returns (in_maps, trivial_gb)."""
    x = np.asarray(inputs["x"], np.float32)
    h = np.asarray(inputs["h_prev"], np.float32)
    memory = np.asarray(inputs["memory"], np.float32)
    gamma = np.asarray(inputs["gamma"], np.float32)
    beta = np.asarray(inputs["beta"], np.float32)
    trivial_gb = bool(np.all(gamma == 1.0) and np.all(beta == 0.0))

    # W/U: w[p, k, n] = W[n, 128k+p] * S in fp8
    wxu = np.stack(
        [
            _to_kxp(np.asarray(inputs[n], np.float32) * WSCALE, f8np)
            for n in ("Ww", "Uw")
        ]
    )
    # Qr/Ql: w[p, c, b, n] = Q[n, 2*(128c+p)+b] * S in fp8
    wq = np.zeros((2, 128, 4, 2, H), f8np)
    qsum = np.zeros(H, np.float32)
    for s, name in enumerate(("Qrw", "Qlw")):
        q = np.asarray(inputs[name], np.float32) * WSCALE  # [out, in]
        q8 = q.astype(f8np)
        qsum += q8.astype(np.float32).sum(axis=1)
        q8v = q8.reshape(H, 4, 128, 2)  # [n, c, p, b]
        wq[s] = np.ascontiguousarray(q8v.transpose(2, 1, 3, 0))
    mw = _to_kxp(np.asarray(inputs["Mw"], np.float32), np.float32)  # [128, KC, 28]

    pw = np.zeros((2 * NB, 1), np.float32)
    pw[:NB, 0] = 2.0 ** np.arange(NB - 1, -1, -1)
    pw2 = np.zeros((2 * NB, 2), np.float32)
    pw2[:NB, 0] = pw[:NB, 0]
    pw2[NB:, 1] = pw[:NB, 0]
    clip = np.array([[0.0, MEM // 2 - 1], [MEM // 2, MEM - 1]], np.float32)

    mem8 = (memory - 0.5).astype(f8np)
    bias = (
        np.asarray(inputs["Wb"], np.float32)
        + np.asarray(inputs["Ub"], np.float32)
        + np.asarray(inputs["Qrb"], np.float32)
        + np.asarray(inputs["Qlb"], np.float32)
    ) * WSCALE + 0.5 * qsum

    const = np.zeros((128, 1252), np.float32)
    const[:, 0:224] = mw.reshape(128, 224)
    const[:, 224:1248] = np.broadcast_to(bias, (128, H))
    const[:2, 1248:1250] = clip
    const[: 2 * NB, 1250:1251] = -np.asarray(inputs["Mb"], np.float32).reshape(
        2 * NB, 1
    )
    const[: 2 * NB, 1251:1252] = pw2.astype(bfnp).view(np.float32)[:, 0:1]

    common = dict(wxu_t=wxu, wq_t=wq, const_t=const, mem_t=mem8)
    if not trivial_gb:
        common["gam_t"] = np.ascontiguousarray(np.broadcast_to(gamma, (128, H)))
        common["bet_t"] = np.ascontiguousarray(np.broadcast_to(beta, (128, H)))

    in_maps = []
    for c in range(NCORES):
        xs = x[c * BL : (c + 1) * BL]
        hs = h[c * BL : (c + 1) * BL]
        in_maps.append(
            dict(
                x8_t=_to_kxp(_rev_blocks(xs), f8np),
                h8_t=_to_kxp(_rev_blocks(hs), f8np),
                h32_t=_to_kxp(hs, np.float32),
                **common,
            )
        )
    return in_maps, trivial_gb


def get_nc(trivial_gb):
    key = ("nc", trivial_gb)
    if key not in _CACHE:
        _CACHE[key] = _build(trivial_gb)
    return _CACHE[key]


def run(inputs, trace=False, **kw):
    in_maps, trivial_gb = prep(inputs)
    nc = get_nc(trivial_gb)
    res = run_bass_kernel_spmd(
        nc, in_maps, core_ids=list(range(NCORES)), trace=trace, **kw
    )
    outs = []
    for c in range(NCORES):
        o = np.asarray(res.results[c]["out_t"]).astype(np.float32)
        outs.append(o.reshape(MT, 128, H)[:, ::-1, :].reshape(BL, H))
    return np.concatenate(outs, axis=0), res


def kernel(**inputs):
    return run(inputs)[0]


# revision 11
# speedup vs baseline: 1.7938x; 1.0788x over previous
"""Trainium2 Bass kernel for nn_BinaryMemoryRNN (scatter_memory).

Computation (reference):
    logits = h_prev @ Mw.T + Mb                 # [B, 28]
    b1/b2  = bits of logits halves (> 0)
    idx1   = clip(sum(b1 * 2^(13-j)), 0, 8191)
    idx2   = clip(sum(b2 * 2^(13-j)), 8192, 16383)
    pre    = x @ Ww.T + h_prev @ Uw.T + mem[idx1] @ Qrw.T + mem[idx2] @ Qlw.T + b
    out    = sigmoid(layernorm(pre) * gamma + beta)

Strategy: data-parallel over batch across 8 cores (1024 rows each).
  - All four big matmuls in fp8 (e4m3) with DoubleRow (2x PE throughput).
    Weights scaled by 512 (LayerNorm is scale-invariant; bias scaled too).
  - Memory table stored centered (mem - 0.5) in fp8; the 0.5*rowsum(Q)
    correction is folded into the bias. Rows are fetched with
    gpsimd.dma_gather(transpose=True), which lands them 16-bit-granular
    interleaved in [feature, batch] layout; the byte-interleaved pairs are
    consumed directly by DoubleRowSwInterleave matmuls (no PE transposes).
    SWInterleave reads stationary columns reversed; this is absorbed by
    staging x8/h8 with each 128-batch block reversed on the host and
    un-reversing the output on the host.
  - logits via split-bf16 (h16@MwHi + h16@MwLo + r16@MwHi) — ~3x faster than
    the quarter-rate fp32 matmul, index-exact vs fp32.
  - The whole index pipeline (logits/bits/idx/wrap/gather) is split by batch
    half so the first gathers issue ~15us earlier.
  - Epilogue: bias add + bn_stats on DVE, rstd via quake-rsqrt bit trick on
    DVE (avoids ACT sqrt<->sigmoid table thrash), single fused
    scale+bias+Sigmoid on ACT, bf16 output.
  - PE HAM warmup matmuls at t0 so the index pipeline runs at 2.4 GHz.
"""

import sys

sys.path.insert(0, "/opt/trn_rl_repo")

from contextlib import ExitStack

import numpy as np
import ml_dtypes

import concourse.bass as bass
import concourse.tile as tile
from concourse import bacc, mybir, library_config
from concourse.bass_utils import run_bass_kernel_spmd

F32 = mybir.dt.float32
BF16 = mybir.dt.bfloat16
F8 = mybir.dt.float8e4
I16 = mybir.dt.int16
I32 = mybir.dt.int32
f8np = ml_dtypes.float8_e4m3fn
bfnp = ml_dtypes.bfloat16

B, I, H, NB = 8192, 1024, 1024, 14
MEM = 2**NB
NCORES = 8
BL = B // NCORES  # 1024 batch rows per core
HB = BL // 2  # 512-row batch half
KC = H // 128  # 8 contraction chunks of 128
MT = BL // 128  # 8 output row-tiles per core
EPS = 1e-5
WSCALE = 512.0
EPS_S = EPS * WSCALE * WSCALE
QUAKE = 0x5F3759DF

_CACHE = {}


def _build(trivial_gb: bool):
    nc = bacc.Bacc(
        "TRN2", target_bir_lowering=False, debug=False, enable_asserts=True
    )

    # activations: feat-major [128, KC, BL]; x8/h8 batch-reversed per 128-block
    x8_t = nc.dram_tensor("x8_t", [128, KC, BL], F8, kind="ExternalInput").ap()
    h8_t = nc.dram_tensor("h8_t", [128, KC, BL], F8, kind="ExternalInput").ap()
    # h split for exact-index logits: per batch-half bf16 high + residual
    hr_t = [
        nc.dram_tensor(f"hr{i}_t", [128, KC, 2, HB], BF16, kind="ExternalInput").ap()
        for i in range(2)
    ]
    # W/U weights: [2, 128, KC, H] fp8, w[s][p,k,n] = Ws[n, 128k+p]*S
    wxu_t = nc.dram_tensor("wxu_t", [2, 128, KC, H], F8, kind="ExternalInput").ap()
    # Qr/Ql weights: [2, 128, 4, 2, H] fp8, w[s][p,c,b,n] = Qs[n, 2*(128c+p)+b]*S
    wq_t = nc.dram_tensor("wq_t", [2, 128, 4, 2, H], F8, kind="ExternalInput").ap()
    # critical consts: mw-bf16-pairs[0:224] | clip[224:226] | negmb[226:227] |
    # pw-as-bf16[227:228]
    NCC = 228
    constc_t = nc.dram_tensor("constc_t", [128, NCC], F32, kind="ExternalInput").ap()
    bias_t = nc.dram_tensor("bias_t", [128, H], F32, kind="ExternalInput").ap()
    mem_t = nc.dram_tensor("mem_t", [MEM, H], F8, kind="ExternalInput").ap()
    if not trivial_gb:
        gam_t = nc.dram_tensor("gam_t", [128, H], F32, kind="ExternalInput").ap()
        bet_t = nc.dram_tensor("bet_t", [128, H], F32, kind="ExternalInput").ap()
    out_t = nc.dram_tensor("out_t", [BL, H], BF16, kind="ExternalOutput").ap()

    DR = mybir.MatmulPerfMode.DoubleRow
    DRI = mybir.MatmulPerfMode.DoubleRowSwInterleave

    with tile.TileContext(nc) as tc:
        with ExitStack() as ctx:
            cpool = ctx.enter_context(tc.tile_pool(name="consts", bufs=1))
            apool = ctx.enter_context(tc.tile_pool(name="acts", bufs=1))
            spool = ctx.enter_context(tc.tile_pool(name="small", bufs=2))
            epool = ctx.enter_context(tc.tile_pool(name="epilogue", bufs=2))
            pp_main = ctx.enter_context(
                tc.tile_pool(name="psum_main", bufs=3, space="PSUM")
            )
            pp_small = ctx.enter_context(
                tc.tile_pool(name="psum_small", bufs=2, space="PSUM")
            )

            nc.gpsimd.load_library(library_config.attnmlp)

            eps_sb = cpool.tile([128, 2], F32, tag="eps")
            nc.vector.memset(eps_sb[:, 0:1], EPS_S)
            nc.vector.memset(eps_sb[:, 1:2], 0.0)
            # prefetch the sigmoid table set while DMAs run
            dum_sb = cpool.tile([128, 1], F32, tag="dum")
            nc.vector.memset(dum_sb[:], 0.0)
            nc.scalar.activation(
                dum_sb[:], dum_sb[:], mybir.ActivationFunctionType.Sigmoid
            )

            # ---------------- input loads (one HWDGE queue, in priority order)
            constc_sb = cpool.tile([128, NCC], F32, tag="constc")
            nc.sync.dma_start(constc_sb[:], constc_t[:])
            mw_bf = constc_sb[:, 0:224].bitcast(BF16).rearrange(
                "p (k j t) -> p k j t", j=2 * NB, t=2
            )
            clip_sb = constc_sb[0:2, 224:226]
            negmb_sb = constc_sb[0 : 2 * NB, 226:227]
            pw_sb = constc_sb[0 : 2 * NB, 227:228].bitcast(BF16)

            hr_sb = []
            for i in range(2):
                hr = spool.tile([128, KC, 2, HB], BF16, tag=f"hr{i}", bufs=1)
                nc.sync.dma_start(hr[:], hr_t[i][:])
                hr_sb.append(hr)
                if i == 0:
                    x8_sb = apool.tile([128, KC, BL], F8, tag="x8")
                    nc.sync.dma_start(x8_sb[:], x8_t[:])
                    h8_sb = apool.tile([128, KC, BL], F8, tag="h8")
                    nc.sync.dma_start(h8_sb[:], h8_t[:])
                    wxu_sb = []
                    for s in range(2):
                        w = cpool.tile([128, KC, H], F8, tag=f"wxu{s}")
                        nc.sync.dma_start(w[:], wxu_t[s])
                        wxu_sb.append(w)
            wq_sb = []
            for s in range(2):
                w = cpool.tile([128, 4, 2, H], F8, tag=f"wq{s}")
                nc.sync.dma_start(w[:], wq_t[s])
                wq_sb.append(w)
            bias_sb = cpool.tile([128, H], F32, tag="bias")
            nc.sync.dma_start(bias_sb[:], bias_t[:])
            if not trivial_gb:
                gam_sb = cpool.tile([128, H], F32, tag="gam")
                nc.sync.dma_start(gam_sb[:], gam_t[:])
                bet_sb = cpool.tile([128, H], F32, tag="bet")
                nc.sync.dma_start(bet_sb[:], bet_t[:])

            # ---------------- PE HAM warmup (junk matmuls on const bits)
            warm_rhs = constc_sb[:, 64:192].bitcast(BF16)  # [128, 256]
            warm_lhs = constc_sb[:, 0:64].bitcast(BF16)  # [128, 128]
            for w in range(24):
                wps = pp_main.tile([128, 512], F32, tag="acc", name=f"wm{w}")
                nc.tensor.matmul(
                    wps[:, 0:256], warm_lhs, warm_rhs, start=True, stop=True,
                )

            # ---------------- per-half index pipeline ----------------
            bits_sb = spool.tile([2 * NB, BL], BF16, tag="bits", bufs=1)
            idx16 = spool.tile([2, BL], I16, tag="idx16", bufs=1)
            g_tiles = [[None, None], [None, None]]  # [r][hf]

            def emit_idx_half(hf):
                hsl = slice(hf * HB, (hf + 1) * HB)
                # logits.T [28, HB]: bf16 split — h16@MwHi + h16@MwLo + r16@MwHi
                lg = pp_small.tile([2 * NB, HB], F32, tag="sm", name=f"lg{hf}")
                first, last = (0, 0), (2, KC - 1)
                for g, (hi, mt) in enumerate(((0, 0), (0, 1), (1, 0))):
                    for k in range(KC):
                        nc.tensor.matmul(
                            lg[:],
                            mw_bf[:, k, :, mt],
                            hr_sb[hf][:, k, hi, :],
                            start=((g, k) == first),
                            stop=((g, k) == last),
                        )
                nc.vector.tensor_scalar(
                    bits_sb[:, hsl], lg[:], negmb_sb[:, 0:1], None,
                    mybir.AluOpType.is_gt,
                )
                ix = pp_small.tile([2, HB], F32, tag="sm", name=f"ix{hf}")
                nc.tensor.matmul(
                    ix[:], pw_sb, bits_sb[:, hsl], start=True, stop=True
                )
                nc.vector.tensor_scalar(
                    idx16[:, hsl], ix[:], clip_sb[:, 0:1], clip_sb[:, 1:2],
                    mybir.AluOpType.max, mybir.AluOpType.min,
                )
                # wrap + gather per index row
                for r in range(2):
                    stg = spool.tile([32, 32], I16, tag="stage")
                    with nc.allow_non_contiguous_dma(reason="tiny idx wrap"):
                        nc.scalar.dma_start(
                            stg[0:32, 0:16],
                            idx16[r : r + 1, hsl].rearrange(
                                "p (a b) -> p a b", b=16
                            ),
                        )
                    nc.vector.tensor_copy(stg[0:32, 16:32], stg[0:32, 0:16])
                    idxw = spool.tile([128, 32], I16, tag="idxw")
                    for g in range(4):
                        nc.vector.transpose(idxw[32 * g : 32 * (g + 1), :], stg[:])
                    g8 = spool.tile([128, 8, HB], F8, tag=f"g{r}{hf}", bufs=1)
                    nc.gpsimd.dma_gather(
                        out_ap=g8[:],
                        in_ap=mem_t[:],
                        idxs_ap=idxw[:],
                        num_idxs=HB,
                        num_idxs_reg=HB,
                        elem_size=H,
                        transpose=True,
                    )
                    g_tiles[r][hf] = g8[:].rearrange("p (c t) i -> p c (t i)", t=2)

            # ---------------- main matmuls + epilogue ----------------
            ps_tiles = {}

            def emit_xh(m):
                ps = pp_main.tile([128, H], F32, tag="acc", name=f"acc{m}")
                ps_tiles[m] = ps
                ms = slice(m * 128, (m + 1) * 128)
                for si, act in enumerate((x8_sb, h8_sb)):
                    for kp in range(KC // 2):
                        lhsT = act[:, 2 * kp : 2 * kp + 2, ms]
                        for n in range(2):
                            nc.tensor.matmul(
                                ps[:, n * 512 : (n + 1) * 512],
                                lhsT,
                                wxu_sb[si][:, 2 * kp : 2 * kp + 2,
                                           n * 512 : (n + 1) * 512],
                                start=(si == 0 and kp == 0),
                                stop=False,
                                perf_mode=DR,
                            )

            def emit_mem_epilogue(m):
                ps = ps_tiles.pop(m)
                ms = slice(m * 128, (m + 1) * 128)
                mm = m % 4
                for si in range(2):
                    g_v = g_tiles[si][m // 4]
                    for c in range(4):
                        lhsT = g_v[:, c, 2 * 128 * mm : 2 * 128 * (mm + 1)]
                        for n in range(2):
                            nc.tensor.matmul(
                                ps[:, n * 512 : (n + 1) * 512],
                                lhsT,
                                wq_sb[si][:, c, :, n * 512 : (n + 1) * 512],
                                start=False,
                                stop=(si == 1 and c == 3),
                                perf_mode=DRI,
                            )

                # t = pre + bias (bias varies along the free/feature dim)
                t = epool.tile([128, H], BF16, tag="t")
                nc.vector.tensor_tensor(
                    t[:], ps[:], bias_sb[:], mybir.AluOpType.add
                )
                # layernorm stats
                st6 = epool.tile([128, 2, 6], F32, tag="st6")
                for a in range(2):
                    nc.vector.bn_stats(st6[:, a, :], t[:, a * 512 : (a + 1) * 512])
                mv = epool.tile([128, 2], F32, tag="mv")
                nc.vector.bn_aggr(mv[:], st6.rearrange("p a b -> p (a b)"))
                # rstd = 1/sqrt(var + eps) via quake bit trick + 1 Newton
                # (max rel err ~1.8e-3; all on DVE, no ACT table swap)
                sc = epool.tile([128, 4], F32, tag="sc")
                v = sc[:, 0:1]
                nc.vector.tensor_scalar(
                    v, mv[:, 1:2], eps_sb[:, 0:1], None, mybir.AluOpType.add
                )
                y0i = sc[:, 1:2].bitcast(I32)
                nc.vector.tensor_scalar(
                    y0i, v.bitcast(I32), 1, None,
                    mybir.AluOpType.logical_shift_right,
                )
                nc.vector.tensor_scalar(
                    y0i, y0i, -1, QUAKE,
                    mybir.AluOpType.mult, mybir.AluOpType.add,
                )
                y0 = sc[:, 1:2]
                a_t = sc[:, 2:3]
                nc.vector.tensor_tensor(a_t, y0, y0, mybir.AluOpType.mult)
                nc.vector.tensor_tensor(a_t, a_t, v, mybir.AluOpType.mult)
                nc.vector.tensor_scalar(
                    a_t, a_t, -0.5, 1.5, mybir.AluOpType.mult, mybir.AluOpType.add
                )
                rstd = sc[:, 3:4]
                nc.vector.tensor_tensor(rstd, y0, a_t, mybir.AluOpType.mult)
                # nmu = -mu * rstd
                nmu = sc[:, 1:2]
                nc.vector.tensor_scalar(
                    nmu, mv[:, 0:1], rstd, -1.0,
                    mybir.AluOpType.mult, mybir.AluOpType.mult,
                )
                o = epool.tile([128, H], BF16, tag="o")
                if trivial_gb:
                    nc.scalar.activation(
                        o[:], t[:], mybir.ActivationFunctionType.Sigmoid,
                        bias=nmu, scale=rstd,
                    )
                else:
                    xh = epool.tile([128, H], F32, tag="xh")
                    nc.scalar.activation(
                        xh[:], t[:], mybir.ActivationFunctionType.Identity,
                        bias=nmu, scale=rstd,
                    )
                    nc.vector.tensor_tensor(
                        xh[:], xh[:], gam_sb[:], mybir.AluOpType.mult
                    )
                    nc.vector.tensor_tensor(
                        xh[:], xh[:], bet_sb[:], mybir.AluOpType.add
                    )
                    nc.scalar.activation(
                        o[:], xh[:], mybir.ActivationFunctionType.Sigmoid,
                        bias=eps_sb[:, 1:2],
                    )
                nc.sync.dma_start(out_t[ms, :], o[:])

            emit_idx_half(0)
            emit_xh(0)
            emit_xh(1)
            emit_idx_half(1)
            emit_xh(2)
            emit_mem_epilogue(0)
            for m in range(3, MT):
                emit_xh(m)
                emit_mem_epilogue(m - 2)
            emit_mem_epilogue(MT - 2)
            emit_mem_epilogue(MT - 1)

    nc.compile()
    return nc


def _to_kxp(a, dtype):
    """[batch, feat] -> [128, KC, batch] with feat = k*128 + p."""
    t = np.ascontiguousarray(a.T.reshape(KC, 128, -1).transpose(1, 0, 2))
    return t.astype(dtype)


def _rev_blocks(a):
    """Reverse each 128-row block along the batch dim of [batch, feat]."""
    return np.ascontiguousarray(
        a.reshape(-1, 128, a.shape[-1])[:, ::-1, :].reshape(a.shape)
    )


def prep(inputs):
    """Host-side shard/layout prep. Returns (in_maps, trivial_gb)."""
    x = np.asarray(inputs["x"], np.float32)
    h = np.asarray(inputs["h_prev"], np.float32)
    memory = np.asarray(inputs["memory"], np.float32)
    gamma = np.asarray(inputs["gamma"], np.float32)
    beta = np.asarray(inputs["beta"], np.float32)
    trivial_gb = bool(np.all(gamma == 1.0) and np.all(beta == 0.0))

    # W/U: w[p, k, n] = W[n, 128k+p] * S in fp8
    wxu = np.stack(
        [
            _to_kxp(np.asarray(inputs[n], np.float32) * WSCALE, f8np)
            for n in ("Ww", "Uw")
        ]
    )
    # Qr/Ql: w[p, c, b, n] = Q[n, 2*(128c+p)+b] * S in fp8
    wq = np.zeros((2, 128, 4, 2, H), f8np)
    qsum = np.zeros(H, np.float32)
    for s, name in enumerate(("Qrw", "Qlw")):
        q = np.asarray(inputs[name], np.float32) * WSCALE  # [out, in]
        q8 = q.astype(f8np)
        qsum += q8.astype(np.float32).sum(axis=1)
        q8v = q8.reshape(H, 4, 128, 2)  # [n, c, p, b]
        wq[s] = np.ascontiguousarray(q8v.transpose(2, 1, 3, 0))

    # Mw split into bf16 high/low pairs packed as f32 columns
    mw = _to_kxp(np.asarray(inputs["Mw"], np.float32), np.float32)  # [128,KC,28]
    mwb = mw.astype(bfnp)
    mws = (mw - mwb.astype(np.float32)).astype(bfnp)
    mwhl = np.stack([mwb, mws], axis=-1)  # [128, KC, 28, 2] bf16

    pw2 = np.zeros((2 * NB, 2), np.float32)
    pw2[:NB, 0] = 2.0 ** np.arange(NB - 1, -1, -1)
    pw2[NB:, 1] = pw2[:NB, 0]
    clip = np.array([[0.0, MEM // 2 - 1], [MEM // 2, MEM - 1]], np.float32)

    mem8 = (memory - 0.5).astype(f8np)
    bias = (
        np.asarray(inputs["Wb"], np.float32)
        + np.asarray(inputs["Ub"], np.float32)
        + np.asarray(inputs["Qrb"], np.float32)
        + np.asarray(inputs["Qlb"], np.float32)
    ) * WSCALE + 0.5 * qsum

    constc = np.zeros((128, 228), np.float32)
    constc[:, 0:224] = mwhl.reshape(128, 448).view(np.float32)
    constc[:2, 224:226] = clip
    constc[: 2 * NB, 226:227] = -np.asarray(inputs["Mb"], np.float32).reshape(
        2 * NB, 1
    )
    constc[: 2 * NB, 227:228] = pw2.astype(bfnp).view(np.float32)[:, 0:1]
    bias128 = np.ascontiguousarray(np.broadcast_to(bias, (128, H)), np.float32)

    common = dict(
        wxu_t=wxu, wq_t=wq, constc_t=constc, bias_t=bias128, mem_t=mem8
    )
    if not trivial_gb:
        common["gam_t"] = np.ascontiguousarray(np.broadcast_to(gamma, (128, H)))
        common["bet_t"] = np.ascontiguousarray(np.broadcast_to(beta, (128, H)))

    in_maps = []
    for c in range(NCORES):
        xs = x[c * BL : (c + 1) * BL]
        hs = h[c * BL : (c + 1) * BL]
        m = dict(
            x8_t=_to_kxp(_rev_blocks(xs), f8np),
            h8_t=_to_kxp(_rev_blocks(hs), f8np),
            **common,
        )
        for i in range(2):
            hh = hs[i * HB : (i + 1) * HB]
            hh16 = hh.astype(bfnp).astype(np.float32)
            h16 = _to_kxp(hh, bfnp)  # [128, KC, HB]
            r16 = _to_kxp(hh - hh16, bfnp)  # residual, bf16
            m[f"hr{i}_t"] = np.ascontiguousarray(
                np.stack([h16, r16], axis=2)
            )  # [128, KC, 2, HB]
        in_maps.append(m)
    return in_maps, trivial_gb


def get_nc(trivial_gb):
    key = ("nc", trivial_gb)
    if key not in _CACHE:
        _CACHE[key] = _build(trivial_gb)
    return _CACHE[key]


def run(inputs, trace=False, **kw):
    in_maps, trivial_gb = prep(inputs)
    nc = get_nc(trivial_gb)
    res = run_bass_kernel_spmd(
        nc, in_maps, core_ids=list(range(NCORES)), trace=trace, **kw
    )
    outs = []
    for c in range(NCORES):
        o = np.asarray(res.results[c]["out_t"]).astype(np.float32)
        outs.append(o.reshape(MT, 128, H)[:, ::-1, :].reshape(BL, H))
    return np.concatenate(outs, axis=0), res


def kernel(**inputs):
    return run(inputs)[0]
